# revision 45
# baseline (speedup 1.0000x reference)
"""Multi-head self-attention (B=2, S=2048, D=1024, H=16, HD=64, causal) on 8 trn2 cores.

Sharding: core c = 4*b + g handles batch b and head group g (4 heads).
  - QKV projections are tensor-parallel over heads (column-split weights).
  - Output projection is row-split over the ctx dims; partial outputs are
    summed on the host (the "all-reduce"), bias added once.

Device kernel design (per core):
  - bf16 matmul operands, fp32 PSUM accumulation. (fp8 would double PE
    throughput via DoubleRow but e4m3's ~3.6% RMS quantization error on
    P/V lands ~5% final rel-err, over the 2e-2 gate.)
  - Scores are computed TRANSPOSED: S^T[k, q] = K_h Q_h^T, so the exp output
    (P^T) is directly the moving operand of the AV matmul - no transposes.
  - Denominators come from a 64-wide ones block PREPENDED to V: the AV matmul
    replicates the softmax denominator across PSUM partitions 0-63 and the
    numerator on 64-127 (free: matmul cycles depend only on the moving
    width). Ones-first keeps every multi-input DVE op in the normalization
    chain on matching base partitions (a walrus constraint).
  - exp without max-subtraction: |scores/8| <= ~3.1 for this input
    distribution, far inside the fp32 exp range.
  - Causal diagonal 128-blocks are masked into separate ptd tiles by gpsimd
    affine_select; the AV is split so only tiny N=128 matmuls depend on the
    masks and the wide AV matmuls chain directly from exp.
  - Normalization is per-head, straight out of PSUM, entirely on the DVE with
    an approximate reciprocal from standard ops (no ACT table swaps - those
    cost 1.28us per Exp<->Reciprocal alternation and the readiness scheduler
    interleaves them per head; no InstReciprocal - 3.4us/tile; no custom-DVE
    ops - they compile but return garbage through this execution path):
      nx = bitcast(~bits(den)); y0 = c0*nx lands 1/den within ~6% for any
      magnitude; one tuned Newton step y1 = y0*(c1 - den*y0) reaches 0.17%.
    Fused as 4 DVE ops (the last one IS the normalize):
      nx = ~den; t = (-c0*den)*nx; z = (c1+t)*nx; ctxn = (c0*num)*z.
    The whole run then needs a single activation-table load (Exp).
  - The AV ctx PSUM tile and the outproj accumulator share one 2-buffer PSUM
    allocation (tag "ctx"), double-buffering both. outproj emission is
    staggered into chunks 2-3 as exp-free PE filler (exp cols per head grow
    ~4c, so the late chunks are where ACT approaches the PE), with drain
    casts on the ACT engine via table-free Copy where ACT has slack.
  - HAM drops the PE clock ~1.9x whenever a 3.4us quantum has <~50% PE busy
    and needs >=~85% to recover - every structural stall multiplies. The DMA
    schedule is staged to keep the PE dense from the start: weight columns +
    chunk-0 x columns per d-tile first (proj(0) chases the DMA), then Wo^T,
    then the remaining x columns. The ones block is memset on the DVE
    instead of DMA'd (8192 tiny descriptors) or gpsimd-memset (gpsimd is
    busy issuing the startup DMA waves and a saturated-gpsimd/idle-PE
    first quantum trips the clock gate).
  - Output is written bf16 (halves the 8.4MB/core output DMA); partials are
    upcast and summed on the host.
"""

import sys

import numpy as np

if "/opt/trn_rl_repo" not in sys.path:
    sys.path.insert(0, "/opt/trn_rl_repo")

B, S, D, H, HD = 2, 2048, 1024, 16, 64
NH = 4          # heads per core
EL = NH * HD    # 256 local projection dims per core
P = 128
NT = S // P     # 16 n-tiles
DTI = D // P    # 8 d-tiles (contraction tiles for projections)
NCH = S // 512  # 4 q-chunks of 512
ET = EL // P    # 2 e-tiles of the local projection dims
VW = 2 * HD     # 128: a 64-wide ones block (denominator) then V

OQ, OK_, OV = S, S + EL, S + 2 * EL
XW = S + 3 * EL  # 2816 columns of the packed input slab

MM_DTYPE = "bfloat16"

# reciprocal-approx constants (Chebyshev-minimax over the bitcast-NOT seed)
C0R = -0.23549792
C1R = 2.0017324

# diagonal-group packing: per chunk, the 4 diagonal k-tiles (j=0..3) keep
# only their valid q-suffix (width 512-128j). j1 (384) and j3 (128) share a
# PSUM bank. offsets within the 1280-wide packed group:
DIAG_OFF = [0, 512, 1024, 896]
DIAG_W = [512, 384, 256, 128]
DIAG_GW = 1280


def build_bass(mm_dtype=MM_DTYPE):
    import concourse.bass as bass  # noqa: F401
    import concourse.mybir as mybir
    import concourse.tile as tile
    from concourse import bacc

    f32 = mybir.dt.float32
    i32 = mybir.dt.int32
    mdt = getattr(mybir.dt, mm_dtype)
    EXP = mybir.ActivationFunctionType.Exp
    GE = mybir.AluOpType.is_ge
    MUL = mybir.AluOpType.mult
    ADD = mybir.AluOpType.add
    NOT = mybir.AluOpType.bitwise_not

    nc = bacc.Bacc("TRN2", target_bir_lowering=False, debug=False, num_devices=8)

    xw_d = nc.dram_tensor("xw", [D, XW], mdt, kind="ExternalInput").ap()
    wot_d = nc.dram_tensor("wot", [EL, D], mdt, kind="ExternalInput").ap()
    out_d = nc.dram_tensor("out", [S, D], mdt, kind="ExternalOutput").ap()

    with tile.TileContext(nc) as tc:
        with (
            tc.tile_pool(name="persist", bufs=1) as persist,
            tc.tile_pool(name="xw", bufs=1) as xw,
            tc.tile_pool(name="ptp", bufs=5) as ptp,
            tc.tile_pool(name="aux", bufs=1) as aux,
            tc.tile_pool(name="osb", bufs=4) as osb,
            tc.tile_pool(name="psb", bufs=1, space="PSUM") as psb,
        ):
            qt = [persist.tile([P, S], mdt, tag=f"qt{e}", name=f"qt{e}")
                  for e in range(ET)]
            kt = [persist.tile([P, S], mdt, tag=f"kt{e}", name=f"kt{e}")
                  for e in range(ET)]
            vaug = [persist.tile([P, NH, VW], mdt, tag=f"va{n}", name=f"va{n}")
                    for n in range(NT)]
            ctxn = [persist.tile([P, S], mdt, tag=f"cx{e}", name=f"cx{e}")
                    for e in range(ET)]
            wot_sb = [persist.tile([P, D], mdt, tag=f"wo{e}", name=f"wo{e}")
                      for e in range(ET)]

            # staged input DMA: proj(0)'s columns first so the PE gets dense
            # fast (HAM clock), then Wo^T (first used ~40us in), then the
            # x columns of chunks 1-3 in use order.
            engs = [nc.sync, nc.scalar, nc.gpsimd]
            xw_sb = [xw.tile([P, XW], mdt, tag=f"xw{dt_}", name=f"xw{dt_}")
                     for dt_ in range(DTI)]
            k = 0
            for dt_ in range(DTI):
                for lo, hi in ((S, S + 512), (0, 512)):
                    engs[k % 3].dma_start(
                        xw_sb[dt_][:, lo:hi], xw_d[P * dt_:P * dt_ + P, lo:hi]
                    )
                    k += 1
            for dt_ in range(DTI):
                engs[k % 3].dma_start(
                    xw_sb[dt_][:, OV:XW], xw_d[P * dt_:P * dt_ + P, OV:XW]
                )
                k += 1
            for e in range(ET):
                engs[k % 3].dma_start(wot_sb[e][:], wot_d[P * e:P * e + P, :])
                k += 1
            for lo, hi in ((512, 1024), (1024, 2048)):
                for dt_ in range(DTI):
                    engs[k % 3].dma_start(
                        xw_sb[dt_][:, lo:hi], xw_d[P * dt_:P * dt_ + P, lo:hi]
                    )
                    k += 1
            # denominator ones blocks: compute-side fill, no DMA. On DVE:
            # gpsimd is saturated at startup issuing wave DMAs, and a
            # 100%-gpsimd/21%-PE first quantum trips the HAM clock gate.
            for n in range(NT):
                nc.vector.memset(vaug[n][:, :, 0:HD], 1.0)

            # sp tiles: [128, 1536] (3 banks), 2 bufs. ctx + pc share 2 more.
            def sp_tile(nm):
                return psb.tile([P, 1536], f32, tag="sp", bufs=2, name=nm)

            def emit_proj(c):
                """Just-in-time projections for chunk c: Q/K columns
                [512c, 512c+512) of both e-tiles plus V n-tiles 4c..4c+3.
                Layout over three sp tiles, one accumulation group per bank:
                A=[Qe0|Ke0|Qe1], B=[Ke1|Vn0|Vn1], C=[Vn2|Vn3|-]."""
                cols = slice(512 * c, 512 * c + 512)
                jobs_per_tile = [
                    [("q", 0), ("k", 0), ("q", 1)],
                    [("k", 1), ("v", 4 * c), ("v", 4 * c + 1)],
                    [("v", 4 * c + 2), ("v", 4 * c + 3)],
                ]
                for ti, jobs in enumerate(jobs_per_tile):
                    sp = sp_tile(f"pj{c}_{ti}")
                    for dt_ in range(DTI):
                        for bi, (kind, idx) in enumerate(jobs):
                            if kind == "v":
                                lhs = xw_sb[dt_][:, P * idx:P * idx + P]
                                rhs = xw_sb[dt_][:, OV:OV + EL]
                                w = EL
                            else:
                                off = OQ if kind == "q" else OK_
                                lhs = xw_sb[dt_][:, off + P * idx:
                                                 off + P * idx + P]
                                rhs = xw_sb[dt_][:, cols]
                                w = 512
                            nc.tensor.matmul(
                                sp[:, 512 * bi:512 * bi + w],
                                lhsT=lhs,
                                rhs=rhs,
                                start=(dt_ == 0),
                                stop=(dt_ == DTI - 1),
                            )
                    for bi, (kind, idx) in enumerate(jobs):
                        if kind == "v":
                            vsrc = sp[:, 512 * bi:512 * bi + EL].rearrange(
                                "p (h w) -> p h w", h=NH
                            )
                            nc.vector.tensor_copy(
                                vaug[idx][:, :, HD:VW], vsrc
                            )
                        else:
                            dst = qt if kind == "q" else kt
                            nc.vector.tensor_copy(
                                dst[idx][:, cols],
                                sp[:, 512 * bi:512 * bi + 512],
                            )

            def emit_st(c, h):
                """scores^T + exp (+ masked diag tiles) for head h, chunk c.

                pt layout: non-diag k-tile kt at [512*kt, 512*kt+512);
                diagonal j at [2048*c + DIAG_OFF[j], +DIAG_W[j]) holding the
                valid q-suffix [128*j, 512). Returns (pt, ptd)."""
                e, off = h // 2, HD * (h % 2)
                pt = ptp.tile([P, 2048 * 3 + DIAG_GW], mdt, tag="pt", name="pt")
                ptd = [
                    ptp.tile([P, P], mdt, tag=f"ptd{j}", bufs=3, name=f"ptd{j}")
                    for j in range(NH)
                ]
                # full-width tiles, groups of 3
                for g0 in range(0, 4 * c, 3):
                    gs = min(3, 4 * c - g0)
                    sp = sp_tile("st")
                    for j in range(gs):
                        kti = g0 + j
                        nc.tensor.matmul(
                            sp[:, 512 * j:512 * j + 512],
                            lhsT=kt[e][off:off + HD, P * kti:P * kti + P],
                            rhs=qt[e][off:off + HD, 512 * c:512 * c + 512],
                            start=True,
                            stop=True,
                        )
                    nc.scalar.activation(
                        pt[:, 512 * g0:512 * (g0 + gs)],
                        sp[:, 0:512 * gs],
                        EXP,
                        scale=0.125,
                    )
                # packed diagonal group: j1 and j3 share a bank (one
                # accumulation group: start on j1, stop on j3).
                sp = sp_tile("std")
                for j, stf in ((0, (True, True)), (1, (True, False)),
                               (3, (False, True)), (2, (True, True))):
                    kti = 4 * c + j
                    q_lo = P * j
                    nc.tensor.matmul(
                        sp[:, DIAG_OFF[j]:DIAG_OFF[j] + DIAG_W[j]],
                        lhsT=kt[e][off:off + HD, P * kti:P * kti + P],
                        rhs=qt[e][off:off + HD,
                                  512 * c + q_lo:512 * c + 512],
                        start=stf[0],
                        stop=stf[1],
                    )
                base = 2048 * c
                nc.scalar.activation(
                    pt[:, base:base + DIAG_GW],
                    sp[:, 0:DIAG_GW],
                    EXP,
                    scale=0.125,
                )
                for j in range(NH):
                    nc.gpsimd.affine_select(
                        out=ptd[j][:],
                        in_=pt[:, base + DIAG_OFF[j]:base + DIAG_OFF[j] + P],
                        pattern=[[1, P]],
                        compare_op=GE,
                        fill=0.0,
                        base=0,
                        channel_multiplier=-1,
                    )
                return pt, ptd

            def emit_av(c, h, pt, ptd):
                nkt = 4 * c + 4
                ctx = psb.tile([P, 512], f32, tag="ctx", bufs=2, name="ctx")
                first = True
                for kti in range(4 * c):
                    nc.tensor.matmul(
                        ctx[:],
                        lhsT=vaug[kti][:, h, :],
                        rhs=pt[:, 512 * kti:512 * kti + 512],
                        start=first,
                        stop=False,
                    )
                    first = False
                base = 2048 * c
                for j in range(NH):
                    kti = 4 * c + j
                    q_lo = P * j
                    if DIAG_W[j] > P:
                        nc.tensor.matmul(
                            ctx[:, q_lo + P:512],
                            lhsT=vaug[kti][:, h, :],
                            rhs=pt[:, base + DIAG_OFF[j] + P:
                                   base + DIAG_OFF[j] + DIAG_W[j]],
                            start=first,
                            stop=False,
                        )
                        first = False
                    nc.tensor.matmul(
                        ctx[:, q_lo:q_lo + P],
                        lhsT=vaug[kti][:, h, :],
                        rhs=ptd[j][:],
                        start=False,
                        stop=(kti == nkt - 1),
                    )
                return ctx

            def emit_norm_head(c, h, ctx):
                """Normalize head h straight out of PSUM on the DVE:
                den = ctx[0:64] (ones-first), num = ctx[64:128].
                y1 = c0*nx*(c1 - den*c0*nx), nx = bitcast(~bits(den));
                the final STT is the normalize itself: (c0*num) * z."""
                e, doff = h // 2, HD * (h % 2)
                den = ctx[0:HD, :]
                nx = aux.tile([HD, 512], f32, tag="nx", bufs=2, name="nx")
                tt = aux.tile([HD, 512], f32, tag="tt", bufs=2, name="tt")
                zz = aux.tile([P, 512], f32, tag="zz", bufs=2, name="zz")
                nc.vector.tensor_scalar(
                    out=nx[:].bitcast(i32),
                    in0=den.bitcast(i32),
                    scalar1=0,
                    scalar2=None,
                    op0=NOT,
                )
                nc.vector.scalar_tensor_tensor(
                    out=tt[:], in0=den, scalar=-C0R, in1=nx[:],
                    op0=MUL, op1=MUL,
                )
                nc.vector.scalar_tensor_tensor(
                    out=zz[HD:P, :], in0=tt[:], scalar=C1R, in1=nx[:],
                    op0=ADD, op1=MUL,
                )
                nc.vector.scalar_tensor_tensor(
                    out=ctxn[e][doff:doff + HD, 512 * c:512 * c + 512],
                    in0=ctx[HD:P, :], scalar=C0R, in1=zz[HD:P, :],
                    op0=MUL, op1=MUL,
                )

            def emit_outproj(c, cast_on_act=False):
                # the tail chunk's drain casts ride the (by then idle) ACT
                # engine via table-free Copy, keeping the DVE clear for the
                # last norm chain
                for nt_ in range(4 * c, 4 * c + 4):
                    for ec in range(2):
                        ps = psb.tile([P, 512], f32, tag="ctx", bufs=2,
                                      name="pc")
                        for e in range(ET):
                            nc.tensor.matmul(
                                ps[:],
                                lhsT=ctxn[e][:, P * nt_:P * nt_ + P],
                                rhs=wot_sb[e][:, 512 * ec:512 * ec + 512],
                                start=(e == 0),
                                stop=(e == ET - 1),
                            )
                        ot = osb.tile([P, 512], mdt, tag="ot", name="ot")
                        if cast_on_act:
                            nc.scalar.activation(
                                ot[:], ps[:],
                                mybir.ActivationFunctionType.Copy,
                            )
                        else:
                            nc.vector.tensor_copy(ot[:], ps[:])
                        nc.sync.dma_start(
                            out_d[P * nt_:P * nt_ + P,
                                  512 * ec:512 * ec + 512],
                            ot[:],
                        )

            # Chunks 0-1 run chunk-major; chunks 2 and 3 interleave
            # head-by-head. Chunk 3 alone is ACT-oversubscribed (exp cols
            # per head grow ~4c: its window needs ~29us of exp against
            # ~26us of PE work, so the PE starves and the HAM gate drops
            # the clock); pairing each c=3 head with a c=2 head plus the
            # staggered outproj/proj filler keeps every quantum PE-bound.
            # outproj emission is deferred into the merged region (the
            # readiness scheduler consumes filler greedily) with drain
            # casts split between the ACT (table-free Copy) and the DVE.
            seq = [(c, h) for c in range(NCH) for h in range(NH)]
            outproj_at = {(2, 1): (0, False), (3, 0): (1, True),
                          (3, 2): (2, True)}
            work = {}
            proj_done = {0}
            emit_proj(0)
            work[seq[0]] = emit_st(*seq[0])
            for i, (c, h) in enumerate(seq):
                if i + 1 < len(seq):
                    nc_, nh_ = seq[i + 1]
                    if nc_ not in proj_done:
                        emit_proj(nc_)
                        proj_done.add(nc_)
                    work[seq[i + 1]] = emit_st(nc_, nh_)
                pt, ptd = work.pop((c, h))
                ctx = emit_av(c, h, pt, ptd)
                emit_norm_head(c, h, ctx)
                if (c, h) in outproj_at:
                    oc, on_act = outproj_at[(c, h)]
                    emit_outproj(oc, cast_on_act=on_act)
            emit_outproj(NCH - 1, cast_on_act=True)

    nc.finalize()
    return nc


def shard_inputs(x, Wq, Wk, Wv, Wo, np_dtype):
    """Build the per-core input maps (host-side resharding)."""
    in_maps = []
    for core in range(8):
        b, g = core // 4, core % 4
        sl = slice(EL * g, EL * g + EL)
        xw = np.concatenate(
            [
                x[b].T.astype(np.float32),
                Wq[sl, :].T.astype(np.float32),
                Wk[sl, :].T.astype(np.float32),
                Wv[sl, :].T.astype(np.float32),
            ],
            axis=1,
        )
        in_maps.append(
            {
                "xw": np.ascontiguousarray(xw.astype(np_dtype)),
                "wot": np.ascontiguousarray(
                    Wo[:, sl].T.astype(np.float32).astype(np_dtype)
                ),
            }
        )
    return in_maps


_CACHE = {}


def kernel(x, Wq, Wk, Wv, Wo, bo, _want_results=False, _trace=False,
           _mm_dtype=MM_DTYPE):
    import concourse.mybir as mybir
    from concourse import bass_utils

    x = np.asarray(x)
    Wq, Wk, Wv, Wo, bo = (np.asarray(a) for a in (Wq, Wk, Wv, Wo, bo))

    key = ("nc", _mm_dtype)
    if key not in _CACHE:
        _CACHE[key] = build_bass(_mm_dtype)
    nc = _CACHE[key]

    np_dtype = mybir.dt.np(getattr(mybir.dt, _mm_dtype))
    in_maps = shard_inputs(x, Wq, Wk, Wv, Wo, np_dtype)
    res = bass_utils.run_bass_kernel_spmd(
        nc, in_maps, core_ids=list(range(8)), trace=_trace
    )

    out = np.zeros((B, S, D), np.float32)
    for core in range(8):
        out[core // 4] += res.results[core]["out"].astype(np.float32)
    out += bo.astype(np.float32)
    if _want_results:
        return out, res
    return out


# revision 46
# speedup vs baseline: 1.0184x; 1.0184x over previous
"""Multi-head self-attention (B=2, S=2048, D=1024, H=16, HD=64, causal) on 8 trn2 cores.

Sharding: core c = 4*b + g handles batch b and head group g (4 heads).
  - QKV projections are tensor-parallel over heads (column-split weights).
  - Output projection is row-split over the ctx dims; partial outputs are
    summed on the host (the "all-reduce"), bias added once.

Device kernel design (per core):
  - bf16 matmul operands, fp32 PSUM accumulation. (fp8 would double PE
    throughput via DoubleRow but e4m3's ~3.6% RMS quantization error on
    P/V lands ~5% final rel-err, over the 2e-2 gate.)
  - Scores are computed TRANSPOSED: S^T[k, q] = K_h Q_h^T, so the exp output
    (P^T) is directly the moving operand of the AV matmul - no transposes.
  - Denominators come from a 64-wide ones block PREPENDED to V: the AV matmul
    replicates the softmax denominator across PSUM partitions 0-63 and the
    numerator on 64-127 (free: matmul cycles depend only on the moving
    width). Ones-first keeps every multi-input DVE op in the normalization
    chain on matching base partitions (a walrus constraint).
  - exp without max-subtraction: |scores/8| <= ~3.1 for this input
    distribution, far inside the fp32 exp range.
  - Causal diagonal 128-blocks are masked into separate ptd tiles by gpsimd
    affine_select; the AV is split so only tiny N=128 matmuls depend on the
    masks and the wide AV matmuls chain directly from exp.
  - Normalization is per-head, straight out of PSUM, entirely on the DVE with
    an approximate reciprocal from standard ops (no ACT table swaps - those
    cost 1.28us per Exp<->Reciprocal alternation and the readiness scheduler
    interleaves them per head; no InstReciprocal - 3.4us/tile; no custom-DVE
    ops - they compile but return garbage through this execution path):
      nx = bitcast(~bits(den)); y0 = c0*nx lands 1/den within ~6% for any
      magnitude; one tuned Newton step y1 = y0*(c1 - den*y0) reaches 0.17%.
    Fused as 4 DVE ops (the last one IS the normalize):
      nx = ~den; t = (-c0*den)*nx; z = (c1+t)*nx; ctxn = (c0*num)*z.
    The whole run then needs a single activation-table load (Exp).
  - The AV ctx PSUM tile and the outproj accumulator share one 2-buffer PSUM
    allocation (tag "ctx"), double-buffering both. outproj emission is
    staggered into chunks 2-3 as exp-free PE filler (exp cols per head grow
    ~4c, so the late chunks are where ACT approaches the PE), with drain
    casts on the ACT engine via table-free Copy where ACT has slack.
  - HAM drops the PE clock ~1.9x whenever a 3.4us quantum has <~50% PE busy
    and needs >=~85% to recover - every structural stall multiplies. The DMA
    schedule is staged to keep the PE dense from the start: weight columns +
    chunk-0 x columns per d-tile first (proj(0) chases the DMA), then Wo^T,
    then the remaining x columns. The ones block is memset on the DVE
    instead of DMA'd (8192 tiny descriptors) or gpsimd-memset (gpsimd is
    busy issuing the startup DMA waves and a saturated-gpsimd/idle-PE
    first quantum trips the clock gate).
  - Output is written bf16 (halves the 8.4MB/core output DMA); partials are
    upcast and summed on the host.
"""

import sys

import numpy as np

if "/opt/trn_rl_repo" not in sys.path:
    sys.path.insert(0, "/opt/trn_rl_repo")

B, S, D, H, HD = 2, 2048, 1024, 16, 64
NH = 4          # heads per core
EL = NH * HD    # 256 local projection dims per core
P = 128
NT = S // P     # 16 n-tiles
DTI = D // P    # 8 d-tiles (contraction tiles for projections)
NCH = S // 512  # 4 q-chunks of 512
ET = EL // P    # 2 e-tiles of the local projection dims
VW = 2 * HD     # 128: a 64-wide ones block (denominator) then V

OQ, OK_, OV = S, S + EL, S + 2 * EL
XW = S + 3 * EL  # 2816 columns of the packed input slab

MM_DTYPE = "bfloat16"

# reciprocal-approx constants (Chebyshev-minimax over the bitcast-NOT seed)
C0R = -0.23549792
C1R = 2.0017324

# diagonal-group packing: per chunk, the 4 diagonal k-tiles (j=0..3) keep
# only their valid q-suffix (width 512-128j). j1 (384) and j3 (128) share a
# PSUM bank. offsets within the 1280-wide packed group:
DIAG_OFF = [0, 512, 1024, 896]
DIAG_W = [512, 384, 256, 128]
DIAG_GW = 1280


def build_bass(mm_dtype=MM_DTYPE):
    import concourse.bass as bass  # noqa: F401
    import concourse.mybir as mybir
    import concourse.tile as tile
    from concourse import bacc

    f32 = mybir.dt.float32
    i32 = mybir.dt.int32
    mdt = getattr(mybir.dt, mm_dtype)
    EXP = mybir.ActivationFunctionType.Exp
    GE = mybir.AluOpType.is_ge
    MUL = mybir.AluOpType.mult
    ADD = mybir.AluOpType.add
    NOT = mybir.AluOpType.bitwise_not

    nc = bacc.Bacc("TRN2", target_bir_lowering=False, debug=False, num_devices=8)

    xw_d = nc.dram_tensor("xw", [D, XW], mdt, kind="ExternalInput").ap()
    wot_d = nc.dram_tensor("wot", [EL, D], mdt, kind="ExternalInput").ap()
    out_d = nc.dram_tensor("out", [S, D], mdt, kind="ExternalOutput").ap()

    with tile.TileContext(nc) as tc:
        with (
            tc.tile_pool(name="persist", bufs=1) as persist,
            tc.tile_pool(name="xw", bufs=1) as xw,
            tc.tile_pool(name="ptp", bufs=5) as ptp,
            tc.tile_pool(name="aux", bufs=1) as aux,
            tc.tile_pool(name="osb", bufs=4) as osb,
            tc.tile_pool(name="psb", bufs=1, space="PSUM") as psb,
        ):
            qt = [persist.tile([P, S], mdt, tag=f"qt{e}", name=f"qt{e}")
                  for e in range(ET)]
            kt = [persist.tile([P, S], mdt, tag=f"kt{e}", name=f"kt{e}")
                  for e in range(ET)]
            vaug = [persist.tile([P, NH, VW], mdt, tag=f"va{n}", name=f"va{n}")
                    for n in range(NT)]
            ctxn = [persist.tile([P, S], mdt, tag=f"cx{e}", name=f"cx{e}")
                    for e in range(ET)]
            wot_sb = [persist.tile([P, D], mdt, tag=f"wo{e}", name=f"wo{e}")
                      for e in range(ET)]

            # staged input DMA: proj(0)'s columns first so the PE gets dense
            # fast (HAM clock), then Wo^T (first used ~40us in), then the
            # x columns of chunks 1-3 in use order.
            engs = [nc.sync, nc.scalar, nc.gpsimd]
            xw_sb = [xw.tile([P, XW], mdt, tag=f"xw{dt_}", name=f"xw{dt_}")
                     for dt_ in range(DTI)]
            k = 0
            for dt_ in range(DTI):
                for lo, hi in ((S, S + 256), (S + 256, S + 512),
                               (0, 256), (256, 512)):
                    engs[k % 3].dma_start(
                        xw_sb[dt_][:, lo:hi], xw_d[P * dt_:P * dt_ + P, lo:hi]
                    )
                    k += 1
            for dt_ in range(DTI):
                engs[k % 3].dma_start(
                    xw_sb[dt_][:, OV:XW], xw_d[P * dt_:P * dt_ + P, OV:XW]
                )
                k += 1
            for e in range(ET):
                engs[k % 3].dma_start(wot_sb[e][:], wot_d[P * e:P * e + P, :])
                k += 1
            for lo, hi in ((512, 1024), (1024, 2048)):
                for dt_ in range(DTI):
                    engs[k % 3].dma_start(
                        xw_sb[dt_][:, lo:hi], xw_d[P * dt_:P * dt_ + P, lo:hi]
                    )
                    k += 1
            # denominator ones blocks: compute-side fill, no DMA. On DVE:
            # gpsimd is saturated at startup issuing wave DMAs, and a
            # 100%-gpsimd/21%-PE first quantum trips the HAM clock gate.
            for n in range(NT):
                nc.vector.memset(vaug[n][:, :, 0:HD], 1.0)

            # sp tiles: [128, 1536] (3 banks), 2 bufs. ctx + pc share 2 more.
            def sp_tile(nm):
                return psb.tile([P, 1536], f32, tag="sp", bufs=2, name=nm)

            def emit_proj(c):
                """Just-in-time projections for chunk c: Q/K columns
                [512c, 512c+512) of both e-tiles plus V n-tiles 4c..4c+3.
                Layout over three sp tiles, one accumulation group per bank:
                A=[Qe0|Ke0|Qe1], B=[Ke1|Vn0|Vn1], C=[Vn2|Vn3|-]."""
                cols = slice(512 * c, 512 * c + 512)
                jobs_per_tile = [
                    [("q", 0), ("k", 0), ("q", 1)],
                    [("k", 1), ("v", 4 * c), ("v", 4 * c + 1)],
                    [("v", 4 * c + 2), ("v", 4 * c + 3)],
                ]
                for ti, jobs in enumerate(jobs_per_tile):
                    sp = sp_tile(f"pj{c}_{ti}")
                    for dt_ in range(DTI):
                        for bi, (kind, idx) in enumerate(jobs):
                            if kind == "v":
                                lhs = xw_sb[dt_][:, P * idx:P * idx + P]
                                rhs = xw_sb[dt_][:, OV:OV + EL]
                                w = EL
                            else:
                                off = OQ if kind == "q" else OK_
                                lhs = xw_sb[dt_][:, off + P * idx:
                                                 off + P * idx + P]
                                rhs = xw_sb[dt_][:, cols]
                                w = 512
                            nc.tensor.matmul(
                                sp[:, 512 * bi:512 * bi + w],
                                lhsT=lhs,
                                rhs=rhs,
                                start=(dt_ == 0),
                                stop=(dt_ == DTI - 1),
                            )
                    for bi, (kind, idx) in enumerate(jobs):
                        if kind == "v":
                            vsrc = sp[:, 512 * bi:512 * bi + EL].rearrange(
                                "p (h w) -> p h w", h=NH
                            )
                            nc.vector.tensor_copy(
                                vaug[idx][:, :, HD:VW], vsrc
                            )
                        else:
                            dst = qt if kind == "q" else kt
                            nc.vector.tensor_copy(
                                dst[idx][:, cols],
                                sp[:, 512 * bi:512 * bi + 512],
                            )

            def emit_st(c, h):
                """scores^T + exp (+ masked diag tiles) for head h, chunk c.

                pt layout: non-diag k-tile kt at [512*kt, 512*kt+512);
                diagonal j at [2048*c + DIAG_OFF[j], +DIAG_W[j]) holding the
                valid q-suffix [128*j, 512). Returns (pt, ptd)."""
                e, off = h // 2, HD * (h % 2)
                pt = ptp.tile([P, 2048 * 3 + DIAG_GW], mdt, tag="pt", name="pt")
                ptd = [
                    ptp.tile([P, P], mdt, tag=f"ptd{j}", bufs=3, name=f"ptd{j}")
                    for j in range(NH)
                ]
                # full-width tiles, groups of 3
                for g0 in range(0, 4 * c, 3):
                    gs = min(3, 4 * c - g0)
                    sp = sp_tile("st")
                    for j in range(gs):
                        kti = g0 + j
                        nc.tensor.matmul(
                            sp[:, 512 * j:512 * j + 512],
                            lhsT=kt[e][off:off + HD, P * kti:P * kti + P],
                            rhs=qt[e][off:off + HD, 512 * c:512 * c + 512],
                            start=True,
                            stop=True,
                        )
                    nc.scalar.activation(
                        pt[:, 512 * g0:512 * (g0 + gs)],
                        sp[:, 0:512 * gs],
                        EXP,
                        scale=0.125,
                    )
                # packed diagonal group: j1 and j3 share a bank (one
                # accumulation group: start on j1, stop on j3).
                sp = sp_tile("std")
                for j, stf in ((0, (True, True)), (1, (True, False)),
                               (3, (False, True)), (2, (True, True))):
                    kti = 4 * c + j
                    q_lo = P * j
                    nc.tensor.matmul(
                        sp[:, DIAG_OFF[j]:DIAG_OFF[j] + DIAG_W[j]],
                        lhsT=kt[e][off:off + HD, P * kti:P * kti + P],
                        rhs=qt[e][off:off + HD,
                                  512 * c + q_lo:512 * c + 512],
                        start=stf[0],
                        stop=stf[1],
                    )
                base = 2048 * c
                nc.scalar.activation(
                    pt[:, base:base + DIAG_GW],
                    sp[:, 0:DIAG_GW],
                    EXP,
                    scale=0.125,
                )
                for j in range(NH):
                    nc.gpsimd.affine_select(
                        out=ptd[j][:],
                        in_=pt[:, base + DIAG_OFF[j]:base + DIAG_OFF[j] + P],
                        pattern=[[1, P]],
                        compare_op=GE,
                        fill=0.0,
                        base=0,
                        channel_multiplier=-1,
                    )
                return pt, ptd

            def emit_av(c, h, pt, ptd):
                nkt = 4 * c + 4
                ctx = psb.tile([P, 512], f32, tag="ctx", bufs=2, name="ctx")
                first = True
                for kti in range(4 * c):
                    nc.tensor.matmul(
                        ctx[:],
                        lhsT=vaug[kti][:, h, :],
                        rhs=pt[:, 512 * kti:512 * kti + 512],
                        start=first,
                        stop=False,
                    )
                    first = False
                base = 2048 * c
                for j in range(NH):
                    kti = 4 * c + j
                    q_lo = P * j
                    if DIAG_W[j] > P:
                        nc.tensor.matmul(
                            ctx[:, q_lo + P:512],
                            lhsT=vaug[kti][:, h, :],
                            rhs=pt[:, base + DIAG_OFF[j] + P:
                                   base + DIAG_OFF[j] + DIAG_W[j]],
                            start=first,
                            stop=False,
                        )
                        first = False
                    nc.tensor.matmul(
                        ctx[:, q_lo:q_lo + P],
                        lhsT=vaug[kti][:, h, :],
                        rhs=ptd[j][:],
                        start=False,
                        stop=(kti == nkt - 1),
                    )
                return ctx

            def emit_norm_head(c, h, ctx):
                """Normalize head h straight out of PSUM on the DVE:
                den = ctx[0:64] (ones-first), num = ctx[64:128].
                y1 = c0*nx*(c1 - den*c0*nx), nx = bitcast(~bits(den));
                the final STT is the normalize itself: (c0*num) * z."""
                e, doff = h // 2, HD * (h % 2)
                den = ctx[0:HD, :]
                nx = aux.tile([HD, 512], f32, tag="nx", bufs=2, name="nx")
                tt = aux.tile([HD, 512], f32, tag="tt", bufs=2, name="tt")
                zz = aux.tile([P, 512], f32, tag="zz", bufs=2, name="zz")
                nc.vector.tensor_scalar(
                    out=nx[:].bitcast(i32),
                    in0=den.bitcast(i32),
                    scalar1=0,
                    scalar2=None,
                    op0=NOT,
                )
                nc.vector.scalar_tensor_tensor(
                    out=tt[:], in0=den, scalar=-C0R, in1=nx[:],
                    op0=MUL, op1=MUL,
                )
                nc.vector.scalar_tensor_tensor(
                    out=zz[HD:P, :], in0=tt[:], scalar=C1R, in1=nx[:],
                    op0=ADD, op1=MUL,
                )
                nc.vector.scalar_tensor_tensor(
                    out=ctxn[e][doff:doff + HD, 512 * c:512 * c + 512],
                    in0=ctx[HD:P, :], scalar=C0R, in1=zz[HD:P, :],
                    op0=MUL, op1=MUL,
                )

            def emit_outproj(c, cast_on_act=False):
                # the tail chunk's drain casts ride the (by then idle) ACT
                # engine via table-free Copy, keeping the DVE clear for the
                # last norm chain
                for nt_ in range(4 * c, 4 * c + 4):
                    for ec in range(2):
                        ps = psb.tile([P, 512], f32, tag="ctx", bufs=2,
                                      name="pc")
                        for e in range(ET):
                            nc.tensor.matmul(
                                ps[:],
                                lhsT=ctxn[e][:, P * nt_:P * nt_ + P],
                                rhs=wot_sb[e][:, 512 * ec:512 * ec + 512],
                                start=(e == 0),
                                stop=(e == ET - 1),
                            )
                        ot = osb.tile([P, 512], mdt, tag="ot", name="ot")
                        if cast_on_act:
                            nc.scalar.activation(
                                ot[:], ps[:],
                                mybir.ActivationFunctionType.Copy,
                            )
                        else:
                            nc.vector.tensor_copy(ot[:], ps[:])
                        nc.sync.dma_start(
                            out_d[P * nt_:P * nt_ + P,
                                  512 * ec:512 * ec + 512],
                            ot[:],
                        )

            # Chunks 0-1 run chunk-major; chunks 2 and 3 interleave
            # head-by-head. Chunk 3 alone is ACT-oversubscribed (exp cols
            # per head grow ~4c: its window needs ~29us of exp against
            # ~26us of PE work, so the PE starves and the HAM gate drops
            # the clock); pairing each c=3 head with a c=2 head plus the
            # staggered outproj/proj filler keeps every quantum PE-bound.
            # outproj emission is deferred into the merged region (the
            # readiness scheduler consumes filler greedily) with drain
            # casts split between the ACT (table-free Copy) and the DVE.
            seq = [(c, h) for c in range(NCH) for h in range(NH)]
            outproj_at = {(2, 1): (0, False), (3, 0): (1, True),
                          (3, 2): (2, True)}
            work = {}
            proj_done = {0}
            emit_proj(0)
            work[seq[0]] = emit_st(*seq[0])
            for i, (c, h) in enumerate(seq):
                if i + 1 < len(seq):
                    nc_, nh_ = seq[i + 1]
                    if nc_ not in proj_done:
                        emit_proj(nc_)
                        proj_done.add(nc_)
                    work[seq[i + 1]] = emit_st(nc_, nh_)
                pt, ptd = work.pop((c, h))
                ctx = emit_av(c, h, pt, ptd)
                emit_norm_head(c, h, ctx)
                if (c, h) in outproj_at:
                    oc, on_act = outproj_at[(c, h)]
                    emit_outproj(oc, cast_on_act=on_act)
            emit_outproj(NCH - 1, cast_on_act=True)

    nc.finalize()
    return nc


def shard_inputs(x, Wq, Wk, Wv, Wo, np_dtype):
    """Build the per-core input maps (host-side resharding)."""
    in_maps = []
    for core in range(8):
        b, g = core // 4, core % 4
        sl = slice(EL * g, EL * g + EL)
        xw = np.concatenate(
            [
                x[b].T.astype(np.float32),
                Wq[sl, :].T.astype(np.float32),
                Wk[sl, :].T.astype(np.float32),
                Wv[sl, :].T.astype(np.float32),
            ],
            axis=1,
        )
        in_maps.append(
            {
                "xw": np.ascontiguousarray(xw.astype(np_dtype)),
                "wot": np.ascontiguousarray(
                    Wo[:, sl].T.astype(np.float32).astype(np_dtype)
                ),
            }
        )
    return in_maps


_CACHE = {}


def kernel(x, Wq, Wk, Wv, Wo, bo, _want_results=False, _trace=False,
           _mm_dtype=MM_DTYPE):
    import concourse.mybir as mybir
    from concourse import bass_utils

    x = np.asarray(x)
    Wq, Wk, Wv, Wo, bo = (np.asarray(a) for a in (Wq, Wk, Wv, Wo, bo))

    key = ("nc", _mm_dtype)
    if key not in _CACHE:
        _CACHE[key] = build_bass(_mm_dtype)
    nc = _CACHE[key]

    np_dtype = mybir.dt.np(getattr(mybir.dt, _mm_dtype))
    in_maps = shard_inputs(x, Wq, Wk, Wv, Wo, np_dtype)
    res = bass_utils.run_bass_kernel_spmd(
        nc, in_maps, core_ids=list(range(8)), trace=_trace
    )

    out = np.zeros((B, S, D), np.float32)
    for core in range(8):
        out[core // 4] += res.results[core]["out"].astype(np.float32)
    out += bo.astype(np.float32)
    if _want_results:
        return out, res
    return out
